# revision 1
# baseline (speedup 1.0000x reference)
"""Trainium2 Bass kernel for nn_Actor (4D strided Minkowski-style conv net + MLP head).

Sharding: 8 cores = batch(4) x X-halves(2) for the heavy input streaming
(pool + conv1 + conv2), then an AllGather of the small conv2 feature maps and
an output-channel sharding (32 och/core) for conv4 so the 105 MB w4 tensor is
read once across the chip instead of once per batch. A second tiny AllGather
replicates the 256-d feature so every core computes the MLP head; host takes
core 0's output.

All convs have kernel == stride (non-overlapping patches), so each conv is a
patch matmul. Host-side numpy only rearranges layout (sharding); all compute
(maxpool, convs, BN, MLP, tanh) runs on-device. BN (inference) is folded into
conv weights host-side, which is pure weight preprocessing.
"""

import sys

sys.path.insert(0, "/opt/trn_rl_repo")

from contextlib import ExitStack

import numpy as np

import concourse.bass as bass
import concourse.tile as tile
from concourse import bacc, mybir
from concourse.bass_utils import run_bass_kernel_spmd

EPS = 1e-5
F32 = mybir.dt.float32
AF = mybir.ActivationFunctionType

# patch-chunk size for the pool+conv1 stream (6400 patches total)
PC = 400
NCHUNK = 6400 // PC
# q-chunk size for conv4 weight streaming (800 q positions total)
QB = 50
NQCH = 800 // QB

LAST_EXEC_NS = None
_CACHE = {}


def _build():
    nc = bacc.Bacc(
        "TRN2",
        target_bir_lowering=False,
        debug=False,
        enable_asserts=False,
        num_devices=8,
    )
    d = {}

    def din(name, shape):
        d[name] = nc.dram_tensor(name, list(shape), F32, kind="ExternalInput").ap()
        return d[name]

    xp1 = din("xp1", (128, 8, 6400))
    xp2 = din("xp2", (34, 8, 6400))
    w1a = din("w1a", (128, 64))
    w1b = din("w1b", (34, 64))
    b1 = din("b1", (64, 1))
    w2 = din("w2", (16, 64, 128))
    b2 = din("b2", (128, 1))
    w4 = din("w4", (128, 800, 32))
    b4 = din("b4", (32, 1))
    wl0 = din("wl0", (12, 512))
    wl1 = din("wl1", (128, 512))
    wl2 = din("wl2", (128, 512))
    bl = din("bl", (128, 4))
    wo = din("wo", (4, 128, 6))
    bo = din("bo", (6, 1))
    jp = din("jp", (4, 6))
    jg = din("jg", (4, 6))
    out = nc.dram_tensor("out", [6, 4], F32, kind="ExternalOutput").ap()

    with TileKernel(nc) as tk:
        tk.run(
            xp1, xp2, w1a, w1b, b1, w2, b2, w4, b4,
            wl0, wl1, wl2, bl, wo, bo, jp, jg, out,
        )
    nc.compile()
    return nc


class TileKernel:
    def __init__(self, nc):
        self.nc = nc
        self.tc = tile.TileContext(nc)
        self.ctx = ExitStack()

    def __enter__(self):
        self.tc.__enter__()
        return self

    def __exit__(self, *a):
        self.ctx.close()
        return self.tc.__exit__(*a)

    def run(self, xp1, xp2, w1a, w1b, b1, w2, b2, w4, b4,
            wl0, wl1, wl2, bl, wo, bo, jp, jg, out):
        nc, tc, ctx = self.nc, self.tc, self.ctx
        sync = nc.sync

        const = ctx.enter_context(tc.tile_pool(name="const", bufs=1))
        stream = ctx.enter_context(tc.tile_pool(name="stream", bufs=3))
        pools = ctx.enter_context(tc.tile_pool(name="pools", bufs=2))
        big = ctx.enter_context(tc.tile_pool(name="big", bufs=1))
        w4p = ctx.enter_context(tc.tile_pool(name="w4p", bufs=3))
        ps1 = ctx.enter_context(tc.tile_pool(name="ps1", bufs=2, space="PSUM"))
        ps2 = ctx.enter_context(tc.tile_pool(name="ps2", bufs=1, space="PSUM"))
        ps4 = ctx.enter_context(tc.tile_pool(name="ps4", bufs=1, space="PSUM"))
        psl = ctx.enter_context(tc.tile_pool(name="psl", bufs=1, space="PSUM"))
        pso = ctx.enter_context(tc.tile_pool(name="pso", bufs=1, space="PSUM"))
        dram = ctx.enter_context(tc.tile_pool(name="dram", bufs=1, space="DRAM"))

        # ---- constants into SBUF ----
        w1a_t = const.tile([128, 64], F32); sync.dma_start(w1a_t[:], w1a)
        w1b_t = const.tile([34, 64], F32); sync.dma_start(w1b_t[:], w1b)
        b1_t = const.tile([64, 1], F32); sync.dma_start(b1_t[:], b1)
        w2_t = const.tile([64, 16 * 128], F32)
        sync.dma_start(w2_t[:].rearrange("p (l m) -> p l m", l=16),
                       w2.rearrange("l p m -> p l m"))
        b2_t = const.tile([128, 1], F32); sync.dma_start(b2_t[:], b2)
        b4_t = const.tile([32, 1], F32); sync.dma_start(b4_t[:], b4)
        wl0_t = const.tile([12, 512], F32); sync.dma_start(wl0_t[:], wl0)
        wl1_t = const.tile([128, 512], F32); sync.dma_start(wl1_t[:], wl1)
        wl2_t = const.tile([128, 512], F32); sync.dma_start(wl2_t[:], wl2)
        bl_t = const.tile([128, 4], F32); sync.dma_start(bl_t[:], bl)
        wo_t = const.tile([128, 24], F32)
        sync.dma_start(wo_t[:].rearrange("p (s n) -> p s n", s=4),
                       wo.rearrange("s p n -> p s n"))
        bo_t = const.tile([6, 1], F32); sync.dma_start(bo_t[:], bo)
        jnt_t = const.tile([12, 4], F32)
        for b in range(4):
            sync.dma_start(jnt_t[0:6, b:b + 1], jp[b, :].rearrange("(k o) -> k o", o=1))
            sync.dma_start(jnt_t[6:12, b:b + 1], jg[b, :].rearrange("(k o) -> k o", o=1))

        # ---- stage A: stream x, maxpool(2,2,2), conv1, into h1 ----
        h1 = big.tile([64, 6400], F32)
        for c in range(NCHUNK):
            sl = slice(c * PC, (c + 1) * PC)
            t1 = stream.tile([128, 8 * PC], F32, tag="t1")
            t1v = t1[:].rearrange("p (s f) -> p s f", s=8)
            for s8 in range(8):
                sync.dma_start(t1v[:, s8, :], xp1[:, s8, sl])
            t2 = stream.tile([34, 8 * PC], F32, tag="t2")
            t2v = t2[:].rearrange("p (s f) -> p s f", s=8)
            for s2 in range(2):
                sync.dma_start(t2v[:, 4 * s2:4 * s2 + 4, :],
                               xp2[:, 4 * s2:4 * s2 + 4, sl])

            po1 = self._pooltree(pools, t1, 128, "a")
            po2 = self._pooltree(pools, t2, 34, "b")

            p1 = ps1.tile([64, PC], F32)
            nc.tensor.matmul(p1[:], w1a_t[:], po1[:], start=True, stop=False)
            nc.tensor.matmul(p1[:], w1b_t[:], po2[:], start=False, stop=True)
            nc.scalar.activation(h1[:, sl], p1[:], AF.Relu, bias=b1_t[:, 0:1])

        # ---- stage B: conv2 (16 shifted matmuls over h1) -> h2 [128, 400] ----
        h1v = h1[:].rearrange(
            "p (pt qx lx qy ly qz lz) -> p pt qx lx qy ly qz lz",
            pt=2, qx=5, lx=2, qy=10, ly=2, qz=8, lz=2)
        p2 = ps2.tile([128, PC], F32)
        li = 0
        for lt in range(2):
            for lx in range(2):
                for ly in range(2):
                    for lz in range(2):
                        rhs = h1v[:, lt, :, lx, :, ly, :, lz]
                        nc.tensor.matmul(
                            p2[:], w2_t[:, li * 128:(li + 1) * 128], rhs,
                            start=(li == 0), stop=(li == 15))
                        li += 1
        h2 = big.tile([128, PC], F32, tag="h2")
        nc.scalar.activation(h2[:], p2[:], AF.Relu, bias=b2_t[:, 0:1])

        # ---- AllGather #1: h2 across 8 ranks ----
        ag1_in = dram.tile([128, PC], F32, tag="ag1i")
        ag1_out = dram.tile([1024, PC], F32, tag="ag1o")
        sync.dma_start(ag1_in[:], h2[:])
        nc.gpsimd.collective_compute(
            "AllGather", mybir.AluOpType.bypass,
            replica_groups=[list(range(8))],
            ins=[ag1_in[:].opt()], outs=[ag1_out[:].opt()])

        # h2g: [128 c2, (q=(h,ql)) x batch] with col = q*4 + b
        h2g = big.tile([128, 800 * 4], F32, tag="h2g")
        h2gv = h2g[:].rearrange("p (q b) -> p q b", b=4)
        for b in range(4):
            for h in range(2):
                sync.dma_start(
                    h2gv[:, h * 400:(h + 1) * 400, b],
                    ag1_out[128 * (2 * b + h):128 * (2 * b + h + 1), :])

        # ---- stage C: conv4, och-sharded (this core's 32 channels) ----
        p4 = ps4.tile([32, 4], F32)
        q = 0
        for qc in range(NQCH):
            w4t = w4p.tile([128, QB * 32], F32, tag="w4t")
            w4tv0 = w4t[:].rearrange("p (q o) -> p q o", q=QB)
            hq = QB // 2
            for wqh in range(2):
                sync.dma_start(
                    w4tv0[:, wqh * hq:(wqh + 1) * hq, :],
                    w4[:, qc * QB + wqh * hq:qc * QB + (wqh + 1) * hq, :])
            w4tv = w4t[:].rearrange("p (q o) -> p q o", q=QB)
            for qi in range(QB):
                nc.tensor.matmul(
                    p4[:], w4tv[:, qi, :], h2gv[:, q, :],
                    start=(q == 0), stop=(q == 799))
                q += 1
        o4 = big.tile([32, 4], F32, tag="o4")
        nc.scalar.activation(o4[:], p4[:], AF.Relu, bias=b4_t[:, 0:1])

        # ---- AllGather #2: full 256-d feature to every core ----
        ag2_in = dram.tile([1, 128], F32, tag="ag2i")
        ag2_out = dram.tile([8, 128], F32, tag="ag2o")
        # DRAM flat index 32*b + j  <- sbuf o4[j, b]
        sync.dma_start(ag2_in[:].rearrange("o (b j) -> o b j", b=4).rearrange("o b j -> j o b"),
                       o4[:].rearrange("j (o b) -> j o b", o=1))
        nc.gpsimd.collective_compute(
            "AllGather", mybir.AluOpType.bypass,
            replica_groups=[list(range(8))],
            ins=[ag2_in[:].opt()], outs=[ag2_out[:].opt()])

        # h256s [128, 8]: col 2b   = feature[b, 0:128]
        #                 col 2b+1 = feature[b, 128:256]
        h256 = big.tile([128, 8], F32, tag="h256")
        agf = ag2_out[:].rearrange("r (b j) -> r b j", b=4)
        for b in range(4):
            for half in range(2):
                for rr in range(4):
                    src = agf[4 * half + rr, b, :].rearrange("(j o) -> j o", o=1)
                    sync.dma_start(
                        h256[32 * rr:32 * rr + 32,
                             2 * b + half:2 * b + half + 1], src)

        # ---- MLP head (redundant on every core) ----
        pl = psl.tile([128, 16], F32)
        rhs1 = h256[:].rearrange("p (b half) -> p half b", half=2)
        for s in range(4):
            msl = slice(128 * s, 128 * (s + 1))
            pls = pl[:, 4 * s:4 * s + 4]
            nc.tensor.matmul(pls, wl0_t[:, msl], jnt_t[:], start=True, stop=False)
            nc.tensor.matmul(pls, wl1_t[:, msl], rhs1[:, 0, :], start=False, stop=False)
            nc.tensor.matmul(pls, wl2_t[:, msl], rhs1[:, 1, :], start=False, stop=True)
        hl = big.tile([128, 16], F32, tag="hl")
        for s in range(4):
            nc.scalar.activation(hl[:, 4 * s:4 * s + 4], pl[:, 4 * s:4 * s + 4],
                                 AF.Relu, bias=bl_t[:, s:s + 1])
        po = pso.tile([6, 4], F32)
        for s in range(4):
            nc.tensor.matmul(po[:], wo_t[:, 6 * s:6 * s + 6], hl[:, 4 * s:4 * s + 4],
                             start=(s == 0), stop=(s == 3))
        ot = big.tile([6, 4], F32, tag="ot")
        nc.scalar.activation(ot[:], po[:], AF.Tanh, bias=bo_t[:, 0:1])
        sync.dma_start(out, ot[:])

    def _pooltree(self, pool, t, p, tag):
        """max over the 8 pooled-window slabs: [p, 8, PC] -> [p, PC]."""
        nc = self.nc
        v8 = t[:].rearrange("p (x c f) -> p x c f", x=4, c=2)
        m4 = pool.tile([p, 4 * PC], F32, tag=tag + "4")
        m4v = m4[:].rearrange("p (x f) -> p x f", x=4)
        nc.vector.tensor_tensor(m4v, v8[:, :, 0, :], v8[:, :, 1, :],
                                op=mybir.AluOpType.max)
        v4 = m4[:].rearrange("p (x c f) -> p x c f", x=2, c=2)
        m2 = pool.tile([p, 2 * PC], F32, tag=tag + "2")
        m2v = m2[:].rearrange("p (x f) -> p x f", x=2)
        nc.vector.tensor_tensor(m2v, v4[:, :, 0, :], v4[:, :, 1, :],
                                op=mybir.AluOpType.max)
        v2 = m2[:].rearrange("p (c f) -> p c f", c=2)
        m1 = pool.tile([p, PC], F32, tag=tag + "1")
        nc.vector.tensor_tensor(m1[:], v2[:, 0, :], v2[:, 1, :],
                                op=mybir.AluOpType.max)
        return m1


def _prep(x, jnt_pos, jnt_goal, w1, b1, g1, be1, m1, v1, w2, b2, g2, be2, m2, v2,
          w4, b4, gn, ben, mn, vn, wl, bl, gl, bel, ml, vl, wo, bo):
    """Host-side shard + layout prep. Returns in_maps (list of 8 dicts)."""
    f = np.float32

    def fold(w, b, g, be, m, v):
        s = (g / np.sqrt(v + EPS)).astype(f)
        return (w * s.reshape((-1,) + (1,) * (w.ndim - 1))).astype(f), \
               ((b - m) * s + be).astype(f)

    w1f, b1f = fold(w1, b1, g1, be1, m1, v1)
    w2f, b2f = fold(w2, b2, g2, be2, m2, v2)
    w4f, b4f = fold(w4, b4, gn, ben, mn, vn)
    wlf, blf = fold(wl, bl, gl, bel, ml, vl)

    # conv1 lhsT [162, 64]
    w1k = np.ascontiguousarray(w1f.transpose(1, 2, 3, 4, 5, 0).reshape(162, 64))
    w1a = np.ascontiguousarray(w1k[:128])
    w1b = np.ascontiguousarray(w1k[128:])
    # conv2 lhsT per shift [16, 64, 128]
    w2k = np.ascontiguousarray(
        w2f.transpose(2, 3, 4, 5, 1, 0).reshape(16, 64, 128))
    # conv4 per-core slice [128 c2, 800 q, 32 o]
    w4q = w4f[:, :, 0].transpose(1, 2, 3, 4, 0).reshape(128, 800, 256)
    # MLP
    wlT = np.ascontiguousarray(wlf.T)          # [268, 512]
    wl0 = np.ascontiguousarray(wlT[0:12])
    wl1 = np.ascontiguousarray(wlT[12:140])
    wl2 = np.ascontiguousarray(wlT[140:268])
    bl4 = np.ascontiguousarray(blf.reshape(4, 128).T)   # [128, 4]
    wo4 = np.ascontiguousarray(wo.T.reshape(4, 128, 6))
    bo1 = np.ascontiguousarray(bo.reshape(6, 1).astype(f))

    shared = dict(
        w1a=w1a, w1b=w1b, b1=b1f.reshape(64, 1), w2=w2k, b2=b2f.reshape(128, 1),
        wl0=wl0, wl1=wl1, wl2=wl2, bl=bl4, wo=wo4, bo=bo1,
        jp=np.ascontiguousarray(jnt_pos.astype(f)),
        jg=np.ascontiguousarray(jnt_goal.astype(f)),
    )

    in_maps = []
    for i in range(8):
        b, h = i // 2, i % 2
        xs = x[b, :, :, 60 * h:60 * (h + 1), :, :]
        # [c2,t6,X60,Y120,Z96] -> (c,pt,kt, px,kx,a, py,ky,b2, pz,kz,c2)
        xr = xs.reshape(2, 2, 3, 10, 3, 2, 20, 3, 2, 16, 3, 2)
        xp = xr.transpose(0, 2, 4, 7, 10, 5, 8, 11, 1, 3, 6, 9)
        xp = np.ascontiguousarray(xp.reshape(162, 8, 6400), dtype=f)
        m = dict(shared)
        m["xp1"] = np.ascontiguousarray(xp[:128])
        m["xp2"] = np.ascontiguousarray(xp[128:])
        m["w4"] = np.ascontiguousarray(w4q[:, :, 32 * i:32 * (i + 1)])
        m["b4"] = np.ascontiguousarray(b4f[32 * i:32 * (i + 1)].reshape(32, 1))
        in_maps.append(m)
    return in_maps


def kernel(**inputs):
    global LAST_EXEC_NS
    if "nc" not in _CACHE:
        _CACHE["nc"] = _build()
    nc = _CACHE["nc"]
    in_maps = _prep(**inputs)
    tr = bool(_CACHE.get("trace"))
    kw = {}
    if tr:
        import shutil
        shutil.rmtree("/tmp/ktrace", ignore_errors=True)
        import os as _os
        _os.makedirs("/tmp/ktrace", exist_ok=True)
        kw["tmpdir"] = "/tmp/ktrace"
    res = run_bass_kernel_spmd(nc, in_maps, core_ids=list(range(8)),
                               trace=tr, **kw)
    LAST_EXEC_NS = res.exec_time_ns
    out = res.results[0]["out"]           # [6, 4]
    return np.ascontiguousarray(out.T)    # [4, 6]



# revision 6
# speedup vs baseline: 4.0589x; 4.0589x over previous
"""Trainium2 Bass kernel for nn_Actor (4D strided Minkowski-style conv net + MLP head).

Sharding v2: Z-block sharding. Z=96 splits into exactly 8 conv2-aligned blocks
of 12, so core i processes x[..., 12i:12(i+1)] for ALL 4 batches and locally
produces h2 for its own conv4 q-slice (qz=i). That makes conv4 q-sharded with
zero cross-core exchange before it; a single 4KB AllReduce of the conv4
partials replaces both AllGathers of the batch-sharded scheme.

conv4 streams w4 (the 105MB tensor, och-paged per q) through the PE as the
stationary operand in 200 [128,128] loads while h2 columns move; w4 is
prefetched into SBUF during the x-streaming phase so the tail is compute-only.

All heavy tensors are cast to bf16 host-side (halves HBM traffic; fp32 PSUM
accumulation keeps rel err ~9e-3, tolerance 2e-2). All convs have
kernel == stride, so each conv is a patch matmul. Host-side numpy only
rearranges layout / folds BN into conv weights (pure weight preprocessing).

Per-chunk pipeline: each 320-patch chunk is one merged DMA per input slab
group; pool tree runs on DVE (128-row part) and GpSimd (34-row part); after
every qx-stripe (2 chunks) conv2 + conv4-accumulate run so only AllReduce+MLP
remain after the stream.
"""

import sys

sys.path.insert(0, "/opt/trn_rl_repo")

from contextlib import ExitStack

import ml_dtypes
import numpy as np

import concourse.bass as bass
import concourse.tile as tile
from concourse import bacc, mybir
from concourse.bass_utils import run_bass_kernel_spmd

EPS = 1e-5
F32 = mybir.dt.float32
BF16 = mybir.dt.bfloat16
AF = mybir.ActivationFunctionType
BF = ml_dtypes.bfloat16

PC = 320           # patch columns per stream chunk
NCH = 6400 // PC   # 20 chunks; 2 chunks == 1 qx stripe
NQX = 10           # qx stripes; stripe s covers q = 10s..10s+9

LAST_EXEC_NS = None
_CACHE = {}


def _build():
    nc = bacc.Bacc(
        "TRN2",
        target_bir_lowering=False,
        debug=False,
        enable_asserts=False,
        num_devices=8,
    )
    d = {}

    def din(name, shape, dt=BF16):
        d[name] = nc.dram_tensor(name, list(shape), dt, kind="ExternalInput").ap()
        return d[name]

    xp1 = din("xp1", (128, NCH, 8 * PC))
    xp2 = din("xp2", (34, NCH, 8 * PC))
    w1a = din("w1a", (128, 64))
    w1b = din("w1b", (34, 64))
    b1 = din("b1", (64, 1), F32)
    w2 = din("w2", (64, 16 * 128))
    b2 = din("b2", (128, 1), F32)
    w4 = din("w4", (128, 100 * 256))
    b4 = din("b4", (128, 2), F32)
    wl0 = din("wl0", (12, 512), F32)
    wl1 = din("wl1", (128, 512), F32)
    wl2 = din("wl2", (128, 512), F32)
    bl = din("bl", (128, 4), F32)
    wo = din("wo", (128, 24), F32)
    bo = din("bo", (6, 1), F32)
    jnt = din("jnt", (12, 4), F32)
    out = nc.dram_tensor("out", [6, 4], F32, kind="ExternalOutput").ap()

    with TileKernel(nc) as tk:
        tk.run(xp1, xp2, w1a, w1b, b1, w2, b2, w4, b4,
               wl0, wl1, wl2, bl, wo, bo, jnt, out)
    nc.compile()
    return nc


class TileKernel:
    def __init__(self, nc):
        self.nc = nc
        self.tc = tile.TileContext(nc)
        self.ctx = ExitStack()

    def __enter__(self):
        self.tc.__enter__()
        return self

    def __exit__(self, *a):
        self.ctx.close()
        return self.tc.__exit__(*a)

    def run(self, xp1, xp2, w1a, w1b, b1, w2, b2, w4, b4,
            wl0, wl1, wl2, bl, wo, bo, jnt, out):
        nc, tc, ctx = self.nc, self.tc, self.ctx
        sync = nc.sync

        const = ctx.enter_context(tc.tile_pool(name="const", bufs=1))
        stream = ctx.enter_context(tc.tile_pool(name="stream", bufs=3))
        pools = ctx.enter_context(tc.tile_pool(name="pools", bufs=2))
        big = ctx.enter_context(tc.tile_pool(name="big", bufs=1))
        ps1 = ctx.enter_context(tc.tile_pool(name="ps1", bufs=2, space="PSUM"))
        ps2 = ctx.enter_context(tc.tile_pool(name="ps2", bufs=2, space="PSUM"))
        ps4 = ctx.enter_context(tc.tile_pool(name="ps4", bufs=1, space="PSUM"))
        psl = ctx.enter_context(tc.tile_pool(name="psl", bufs=1, space="PSUM"))
        pso = ctx.enter_context(tc.tile_pool(name="pso", bufs=1, space="PSUM"))
        dram = ctx.enter_context(tc.tile_pool(name="dram", bufs=1, space="DRAM"))

        # ---- constants into SBUF ----
        w1a_t = const.tile([128, 64], BF16); sync.dma_start(w1a_t[:], w1a)
        w1b_t = const.tile([34, 64], BF16); sync.dma_start(w1b_t[:], w1b)
        b1_t = const.tile([64, 1], F32); sync.dma_start(b1_t[:], b1)
        w2_t = const.tile([64, 16 * 128], BF16); sync.dma_start(w2_t[:], w2)
        b2_t = const.tile([128, 1], F32); sync.dma_start(b2_t[:], b2)
        b4_t = const.tile([128, 2], F32); sync.dma_start(b4_t[:], b4)
        wl0_t = const.tile([12, 512], F32); sync.dma_start(wl0_t[:], wl0)
        wl1_t = const.tile([128, 512], F32); sync.dma_start(wl1_t[:], wl1)
        wl2_t = const.tile([128, 512], F32); sync.dma_start(wl2_t[:], wl2)
        bl_t = const.tile([128, 4], F32); sync.dma_start(bl_t[:], bl)
        wo_t = const.tile([128, 24], F32); sync.dma_start(wo_t[:], wo)
        bo_t = const.tile([6, 1], F32); sync.dma_start(bo_t[:], bo)
        jnt_t = const.tile([12, 4], F32); sync.dma_start(jnt_t[:], jnt)

        # w4: prefetched stripe-by-stripe during the stream phase
        w4_t = const.tile([128, 100 * 256], BF16)
        SW = 10 * 256  # cols per stripe

        def w4_fetch(s):
            sync.dma_start(w4_t[:, s * SW:(s + 1) * SW], w4[:, s * SW:(s + 1) * SW])

        w4_fetch(0)
        w4tv = w4_t[:].rearrange("p (q o) -> p q o", q=100)

        h1 = big.tile([64, 6400], BF16)
        # h1 col = qx*640 + qy*64 + pt*32 + lx*16 + ly*8 + lz*4 + b
        h1v = h1[:].rearrange(
            "p (qx qy pt lx ly lz b) -> p qx qy pt lx ly lz b",
            qx=10, qy=10, pt=2, lx=2, ly=2, lz=2)
        h2 = big.tile([128, 400], BF16)       # col = q*4 + b
        h2v = h2[:].rearrange("p (q b) -> p q b", b=4)
        p4 = ps4.tile([128, 8], F32)          # conv4 acc: col = 4*och_half + b

        # ---- streaming phase: pool + conv1 per chunk; conv2+conv4 per stripe ----
        for c in range(NCH):
            if c % 2 == 0 and c // 2 + 1 < NQX:
                w4_fetch(c // 2 + 1)
            t1 = stream.tile([128, 8 * PC], BF16, tag="t1")
            sync.dma_start(t1[:], xp1[:, c, :])
            t2 = stream.tile([34, 8 * PC], BF16, tag="t2")
            sync.dma_start(t2[:], xp2[:, c, :])

            po1 = self._pooltree(nc.vector, pools, t1, 128, "a")
            po2 = self._pooltree(nc.vector, pools, t2, 34, "b")

            p1 = ps1.tile([64, PC], F32)
            nc.tensor.matmul(p1[:], w1a_t[:], po1[:], start=True, stop=False)
            nc.tensor.matmul(p1[:], w1b_t[:], po2[:], start=False, stop=True)
            nc.scalar.activation(h1[:, c * PC:(c + 1) * PC], p1[:],
                                 AF.Relu, bias=b1_t[:, 0:1])

            if c % 2 == 1:
                qx = c // 2
                # conv2 stripe: 16 shifted matmuls -> [128, 40] (qy, b)
                p2 = ps2.tile([128, 40], F32)
                li = 0
                for lt in range(2):
                    for lx in range(2):
                        for ly in range(2):
                            for lz in range(2):
                                rhs = h1v[:, qx, :, lt, lx, ly, lz, :]
                                nc.tensor.matmul(
                                    p2[:], w2_t[:, li * 128:(li + 1) * 128], rhs,
                                    start=(li == 0), stop=(li == 15))
                                li += 1
                nc.scalar.activation(h2[:, qx * 40:(qx + 1) * 40], p2[:],
                                     AF.Relu, bias=b2_t[:, 0:1])
                # conv4 stripe: accumulate q = 10*qx .. 10*qx+9
                for j in range(10):
                    q = qx * 10 + j
                    rhs4 = h2v[:, q, :]
                    for hh in range(2):
                        # NB: start=True resets the accumulate state of the
                        # whole 2KB PSUM zero region, so only the very first
                        # matmul of the group may set it.
                        nc.tensor.matmul(
                            p4[:, 4 * hh:4 * hh + 4],
                            w4tv[:, q, 128 * hh:128 * hh + 128], rhs4,
                            start=(q == 0 and hh == 0), stop=(q == 99 and hh == 1))

        # ---- AllReduce conv4 partials: [128, 8] f32 = 4KB ----
        po4 = big.tile([128, 8], F32, tag="po4")
        nc.scalar.activation(po4[:], p4[:], AF.Copy)
        ar_in = dram.tile([128, 8], F32, tag="ari")
        ar_out = dram.tile([128, 8], F32, tag="aro")
        sync.dma_start(ar_in[:], po4[:])
        nc.gpsimd.collective_compute(
            "AllReduce", mybir.AluOpType.add,
            replica_groups=[list(range(8))],
            ins=[ar_in[:].opt()], outs=[ar_out[:].opt()])
        h256 = big.tile([128, 8], F32, tag="h256")
        sync.dma_start(h256[:], ar_out[:])

        # bias + relu per och half: col = 4*half + b
        h256r = big.tile([128, 8], F32, tag="h256r")
        nc.scalar.activation(h256r[:, 0:4], h256[:, 0:4], AF.Relu, bias=b4_t[:, 0:1])
        nc.scalar.activation(h256r[:, 4:8], h256[:, 4:8], AF.Relu, bias=b4_t[:, 1:2])

        # ---- MLP head (redundant on every core) ----
        pl = psl.tile([128, 16], F32)
        for s in range(4):
            msl = slice(128 * s, 128 * (s + 1))
            pls = pl[:, 4 * s:4 * s + 4]
            nc.tensor.matmul(pls, wl0_t[:, msl], jnt_t[:], start=True, stop=False)
            nc.tensor.matmul(pls, wl1_t[:, msl], h256r[:, 0:4], start=False, stop=False)
            nc.tensor.matmul(pls, wl2_t[:, msl], h256r[:, 4:8], start=False, stop=True)
        hl = big.tile([128, 16], F32, tag="hl")
        for s in range(4):
            nc.scalar.activation(hl[:, 4 * s:4 * s + 4], pl[:, 4 * s:4 * s + 4],
                                 AF.Relu, bias=bl_t[:, s:s + 1])
        po = pso.tile([6, 4], F32)
        wo_v = wo_t[:].rearrange("p (s n) -> p s n", s=4)
        for s in range(4):
            nc.tensor.matmul(po[:], wo_v[:, s, :], hl[:, 4 * s:4 * s + 4],
                             start=(s == 0), stop=(s == 3))
        ot = big.tile([6, 4], F32, tag="ot")
        nc.scalar.activation(ot[:], po[:], AF.Tanh, bias=bo_t[:, 0:1])
        sync.dma_start(out, ot[:])

    def _pooltree(self, eng, pool, t, p, tag):
        """max over the 8 pooled-window slabs: [p, 8, PC] -> [p, PC]."""
        v8 = t[:].rearrange("p (x c f) -> p x c f", x=4, c=2)
        m4 = pool.tile([p, 4 * PC], BF16, tag=tag + "4")
        m4v = m4[:].rearrange("p (x f) -> p x f", x=4)
        eng.tensor_tensor(m4v, v8[:, :, 0, :], v8[:, :, 1, :],
                          op=mybir.AluOpType.max)
        v4 = m4[:].rearrange("p (x c f) -> p x c f", x=2, c=2)
        m2 = pool.tile([p, 2 * PC], BF16, tag=tag + "2")
        m2v = m2[:].rearrange("p (x f) -> p x f", x=2)
        eng.tensor_tensor(m2v, v4[:, :, 0, :], v4[:, :, 1, :],
                          op=mybir.AluOpType.max)
        v2 = m2[:].rearrange("p (c f) -> p c f", c=2)
        m1 = pool.tile([p, PC], BF16, tag=tag + "1")
        eng.tensor_tensor(m1[:], v2[:, 0, :], v2[:, 1, :],
                          op=mybir.AluOpType.max)
        return m1


def _prep(x, jnt_pos, jnt_goal, w1, b1, g1, be1, m1, v1, w2, b2, g2, be2, m2, v2,
          w4, b4, gn, ben, mn, vn, wl, bl, gl, bel, ml, vl, wo, bo):
    """Host-side shard + layout prep (layout/precision only). 8 in_maps."""
    f = np.float32

    def fold(w, b, g, be, m, v):
        s = (g / np.sqrt(v + EPS)).astype(f)
        return (w * s.reshape((-1,) + (1,) * (w.ndim - 1))).astype(f), \
               ((b - m) * s + be).astype(f)

    w1f, b1f = fold(w1, b1, g1, be1, m1, v1)
    w2f, b2f = fold(w2, b2, g2, be2, m2, v2)
    w4f, b4f = fold(w4, b4, gn, ben, mn, vn)
    wlf, blf = fold(wl, bl, gl, bel, ml, vl)

    # conv1 lhsT [162, 64], K order (c, kt, kx, ky, kz)
    w1k = np.ascontiguousarray(
        w1f.transpose(1, 2, 3, 4, 5, 0).reshape(162, 64)).astype(BF)
    # conv2 lhsT per shift: [64, 16*128], shift order (lt, lx, ly, lz)
    w2k = np.ascontiguousarray(
        w2f.transpose(2, 3, 4, 5, 1, 0).reshape(16, 64, 128)
        .transpose(1, 0, 2).reshape(64, 16 * 128)).astype(BF)
    # MLP
    wlT = np.ascontiguousarray(wlf.T)          # [268, 512]
    wl0 = np.ascontiguousarray(wlT[0:12])
    wl1 = np.ascontiguousarray(wlT[12:140])
    wl2 = np.ascontiguousarray(wlT[140:268])
    bl4 = np.ascontiguousarray(blf.reshape(4, 128).T)     # [128, 4]
    wo4 = np.ascontiguousarray(
        wo.T.reshape(4, 128, 6).transpose(1, 0, 2).reshape(128, 24).astype(f))
    bo1 = np.ascontiguousarray(bo.reshape(6, 1).astype(f))
    b4r = np.ascontiguousarray(b4f.reshape(2, 128).T)     # [128, 2]
    jntc = np.ascontiguousarray(
        np.concatenate([jnt_pos, jnt_goal], axis=1).T.astype(f))  # [12, 4]

    shared = dict(
        w1a=np.ascontiguousarray(w1k[:128]),
        w1b=np.ascontiguousarray(w1k[128:]),
        b1=b1f.reshape(64, 1), w2=w2k, b2=b2f.reshape(128, 1),
        b4=b4r, wl0=wl0, wl1=wl1, wl2=wl2, bl=bl4, wo=wo4, bo=bo1, jnt=jntc,
    )

    xb = x.astype(BF)  # one contiguous cast, then per-core strided gathers
    in_maps = []
    for i in range(8):
        xs = xb[:, :, :, :, :, 12 * i:12 * (i + 1)]
        # axes after reshape:
        # b0 c1 pt2 kt3 qx4 lx5 kx6 sx7 qyh8 qyl9 ly10 ky11 sy12 pz13 kz14 sz15
        xr = xs.reshape(4, 2, 2, 3, 10, 2, 3, 2, 2, 5, 2, 3, 2, 2, 3, 2)
        # -> [K(c,kt,kx,ky,kz)=162, chunk(qx,qyh)=20, slab(sx,sy,sz)=8,
        #     incol(qyl,pt,lx,ly,pz,b)=320]
        xp = xr.transpose(1, 3, 6, 11, 14, 4, 8, 7, 12, 15, 9, 2, 5, 10, 13, 0)
        xp = np.ascontiguousarray(xp).reshape(162, 20, 8 * 320)
        # conv4 weights: this core's qz=i slice -> [128c, 100q, 256o]
        w4q = np.ascontiguousarray(
            w4f[:, :, 0, :, :, i].transpose(1, 2, 3, 0).reshape(128, 100 * 256)
        ).astype(BF)
        m = dict(shared)
        m["xp1"] = np.ascontiguousarray(xp[:128])
        m["xp2"] = np.ascontiguousarray(xp[128:])
        m["w4"] = w4q
        in_maps.append(m)
    return in_maps


def kernel(**inputs):
    global LAST_EXEC_NS
    if "nc" not in _CACHE:
        _CACHE["nc"] = _build()
    nc = _CACHE["nc"]
    in_maps = _prep(**inputs)
    tr = bool(_CACHE.get("trace"))
    kw = {}
    if tr:
        import shutil
        shutil.rmtree("/tmp/ktrace", ignore_errors=True)
        import os as _os
        _os.makedirs("/tmp/ktrace", exist_ok=True)
        kw["tmpdir"] = "/tmp/ktrace"
    res = run_bass_kernel_spmd(nc, in_maps, core_ids=list(range(8)),
                               trace=tr, **kw)
    LAST_EXEC_NS = res.exec_time_ns
    out = res.results[0]["out"]           # [6, 4]
    return np.ascontiguousarray(out.T)    # [4, 6]
